# revision 1
# baseline (speedup 1.0000x reference)
"""HAN entailment model on 8 TRN2 NeuronCores.

Strategy:
  - The 8192-step sentence GRU is the critical path. It is computed with
    Picard (fixed-point) iteration over the whole sequence: each iteration is
    a parallel batched matmul [768,256]@[256,N] plus elementwise gates. The
    recurrence is contractive (|dh_t/dh_{t-1}| ~ 0.65), so ~24 iterations
    reach ~1e-4 abs error.  Sequence dim is sharded: each core owns 1024
    positions plus a 32-position halo on the left; the halo absorbs the
    boundary error (decays at the same contraction rate per position), so no
    cross-core communication during iterations.  Core 0's halo rows are
    zero-padded, and an input-augmentation feature forces its z-gate to 1 on
    those rows so h stays exactly 0 through the pad.
  - Biases are folded into the input projection via an extra constant-one
    input feature (bhh_n stays separate: it sits inside the r* product).
  - hs_g (claim-gated states) are all-gathered (8 MB), then each core
    computes its 1024 rows of the [8192,8192] coherence attention with an
    unstabilized softmax (scores are provably < 88, exp is safe in f32).
  - Entailment softmax over dim 0 = one 1.5 KB AllReduce of (sum_i a_i*h_i,
    sum_i a_i).
Layout convention: everything lives feature-on-partitions, positions on the
free dim ("transposed"), so gates/softmax reductions are free-dim native.
"""

import numpy as np

import concourse.bass as bass
import concourse.bacc as bacc
import concourse.tile as tile
import concourse.mybir as mybir
from concourse.bass_utils import run_bass_kernel_spmd

F32 = mybir.dt.float32
BF16 = mybir.dt.bfloat16
AF = mybir.ActivationFunctionType
OP = mybir.AluOpType
AX = mybir.AxisListType

H = 256
E = 300
EP = 384            # padded input features: 300 real + mask(300) + one(301)
LS = 8192
NCORES = 8
SH = LS // NCORES   # kept positions per core
D = 32              # halo
NL = SH + D         # processed positions per core
K_IT = 24           # Picard iterations
CH = 512            # free-dim chunk (f32 matmul max moving free dim)

_built = {}


def _chunks(total, ch=CH):
    out = []
    a = 0
    while a < total:
        out.append((a, min(ch, total - a)))
        a += ch
    return out


def build_nc():
    nc = bacc.Bacc(None, target_bir_lowering=False, debug=False)

    def dp(name, shape):
        return nc.declare_dram_parameter(name, shape, F32, isOutput=False)

    xT_d = dp("xT", [3, 128, NL])
    wihT_d = dp("wihT", [3, 128, 768])
    whhT_d = dp("whhT", [2, 128, 768])
    bhhn_d = dp("bhhn", [128, 2])
    cwihT_d = dp("cwihT", [3, 128, 768])
    claimT_d = dp("claimT", [3, 128, 1])
    cbhhn_d = dp("cbhhn", [128, 2])
    gswT_d = dp("gswT", [2, 128, 1])
    gcwT_d = dp("gcwT", [2, 128, 1])
    awcT_d = dp("awcT", [2, 128, 256])
    acb_d = dp("acb", [128, 2])
    awsT_d = dp("awsT", [2, 128, 1])
    asb_d = dp("asb", [1, 1])
    extWT_d = dp("extWT", [4, 128, 256])
    extb_d = dp("extb", [128, 2])
    jWT_d = dp("jWT", [8, 128, 256])
    entWT_d = dp("entWT", [2, 128, 1])
    entb_d = dp("entb", [1, 1])
    fwT_d = dp("fwT", [2, 128, 3])
    fb_d = dp("fb", [1, 3])
    ident_d = dp("ident", [128, 128])
    out_d = nc.declare_dram_parameter("out", [1, 3], F32, isOutput=True)

    with tile.TileContext(nc) as tc:
        with tc.tile_pool(name="persist", bufs=1) as pp, \
             tc.tile_pool(name="dram", bufs=1, space="DRAM") as dram:
            # ---- persistent SBUF tiles ----
            whhT = pp.tile([128, 2, 768], F32, tag="whhT")
            bhhn = pp.tile([128, 2], F32, tag="bhhn")
            hA = pp.tile([128, 2, NL + 1], F32, tag="hA")
            hB = pp.tile([128, 2, NL + 1], F32, tag="hB")
            hc = pp.tile([128, 2], F32, tag="hc")
            ones_k1 = pp.tile([1, 128], F32, tag="ones_k1")
            ones128 = pp.tile([128, 1], BF16, tag="ones128")
            ident = pp.tile([128, 128], F32, tag="ident")
            aug2 = pp.tile([2, 128], F32, tag="aug2")
            uT = pp.tile([128, 2, SH], F32, tag="uT")
            vpad = pp.tile([2, SH], F32, tag="vpad")

            for kt in range(2):
                nc.sync.dma_start(out=whhT[:, kt, :], in_=whhT_d[kt])
            nc.sync.dma_start(out=bhhn[:], in_=bhhn_d[:, :])
            nc.sync.dma_start(out=ident[:], in_=ident_d[:, :])
            nc.vector.memset(ones_k1[:], 1.0)
            nc.vector.memset(ones128[:], 1.0)
            nc.vector.memset(aug2[:], 0.0)
            nc.vector.memset(aug2[0:1, :], 1.0)
            nc.vector.memset(hA[:], 0.0)
            nc.vector.memset(hB[:], 0.0)

            # =========== claim GRU (single step from h=0) ===========
            with tc.tile_pool(name="cl", bufs=1) as cp, \
                 tc.tile_pool(name="clps", bufs=1, space="PSUM") as cps:
                cwihT = cp.tile([128, 3, 768], F32, tag="cwihT")
                claimT = cp.tile([128, 3, 1], F32, tag="claimT")
                cbhhn = cp.tile([128, 2], F32, tag="cbhhn")
                for kt in range(3):
                    nc.sync.dma_start(out=cwihT[:, kt, :], in_=cwihT_d[kt])
                    nc.sync.dma_start(out=claimT[:, kt, :], in_=claimT_d[kt])
                nc.sync.dma_start(out=cbhhn[:], in_=cbhhn_d[:, :])
                gxc = cps.tile([128, 6], F32, tag="gxc")
                for c in range(6):
                    for kt in range(3):
                        nc.tensor.matmul(
                            gxc[:, c:c + 1],
                            cwihT[:, kt, 128 * c:128 * c + 128],
                            claimT[:, kt, :],
                            start=(kt == 0), stop=(kt == 2),
                        )
                rzc = cp.tile([128, 4], F32, tag="rzc")
                nc.scalar.activation(rzc[:], gxc[:, 0:4], AF.Sigmoid)
                tn = cp.tile([128, 2], F32, tag="tn")
                nn_ = cp.tile([128, 2], F32, tag="nn")
                for c2 in range(2):
                    # (r * bhh_n) + gx_n
                    nc.vector.scalar_tensor_tensor(
                        tn[:, c2:c2 + 1], rzc[:, c2:c2 + 1], cbhhn[:, c2:c2 + 1],
                        gxc[:, 4 + c2:5 + c2], op0=OP.mult, op1=OP.add,
                    )
                nc.scalar.activation(nn_[:], tn[:], AF.Tanh)
                zn = cp.tile([128, 2], F32, tag="zn")
                nc.vector.tensor_tensor(zn[:], rzc[:, 2:4], nn_[:], OP.mult)
                nc.vector.tensor_tensor(hc[:], nn_[:], zn[:], OP.subtract)

            # =========== sentence GRU: gx then Picard iterations ===========
            with tc.tile_pool(name="gru", bufs=1) as gp, \
                 tc.tile_pool(name="gxps", bufs=2, space="PSUM") as gxps:
                xT = gp.tile([128, 3, NL], F32, tag="xT")
                wihT = gp.tile([128, 3, 768], F32, tag="wihT")
                gx = gp.tile([128, 6, NL], F32, tag="gx")
                for kt in range(3):
                    nc.sync.dma_start(out=xT[:, kt, :], in_=xT_d[kt])
                    nc.sync.dma_start(out=wihT[:, kt, :], in_=wihT_d[kt])
                for (a, n) in _chunks(NL):
                    for c in range(6):
                        ps = gxps.tile([128, CH], F32, tag="gxp")
                        for kt in range(3):
                            nc.tensor.matmul(
                                ps[:, :n],
                                wihT[:, kt, 128 * c:128 * c + 128],
                                xT[:, kt, a:a + n],
                                start=(kt == 0), stop=(kt == 2),
                            )
                        nc.scalar.activation(gx[:, c, a:a + n], ps[:, :n], AF.Copy)

                with tc.tile_pool(name="ghps", bufs=1, space="PSUM") as ghps, \
                     tc.tile_pool(name="gsc", bufs=2) as gsc:
                    cur, nxt = hA, hB
                    for k in range(K_IT):
                        for (a, n) in _chunks(NL):
                            ghs = [ghps.tile([128, CH], F32, tag=f"gh{c}", name=f"gh{c}") for c in range(6)]
                            for c in range(6):
                                for c2 in range(2):
                                    nc.tensor.matmul(
                                        ghs[c][:, :n], whhT[:, c2, 128 * c:128 * c + 128],
                                        cur[:, c2, a:a + n],
                                        start=(c2 == 0),
                                        stop=(c2 == 1 and c >= 4),
                                    )
                                if c < 4:
                                    nc.tensor.matmul(
                                        ghs[c][:, :n], ident[:], gx[:, c, a:a + n],
                                        start=False, stop=True,
                                    )
                            for c2 in range(2):
                                r = gsc.tile([128, CH], F32, tag=f"r{c2}")
                                z = gsc.tile([128, CH], F32, tag=f"z{c2}")
                                t1 = gsc.tile([128, CH], F32, tag=f"t1{c2}")
                                t2 = gsc.tile([128, CH], F32, tag=f"t2{c2}")
                                nn2 = gsc.tile([128, CH], F32, tag=f"nn{c2}")
                                dd = gsc.tile([128, CH], F32, tag=f"dd{c2}")
                                ee = gsc.tile([128, CH], F32, tag=f"ee{c2}")
                                nc.scalar.activation(r[:, :n], ghs[0 + c2][:, :n], AF.Sigmoid)
                                nc.scalar.activation(z[:, :n], ghs[2 + c2][:, :n], AF.Sigmoid)
                                nc.vector.scalar_tensor_tensor(
                                    t1[:, :n], ghs[4 + c2][:, :n], bhhn[:, c2:c2 + 1],
                                    r[:, :n], op0=OP.add, op1=OP.mult,
                                )
                                nc.vector.tensor_tensor(t2[:, :n], t1[:, :n], gx[:, 4 + c2, a:a + n], OP.add)
                                nc.scalar.activation(nn2[:, :n], t2[:, :n], AF.Tanh)
                                nc.gpsimd.tensor_tensor(dd[:, :n], cur[:, c2, a:a + n], nn2[:, :n], OP.subtract)
                                nc.gpsimd.tensor_tensor(ee[:, :n], z[:, :n], dd[:, :n], OP.mult)
                                nc.vector.tensor_tensor(nxt[:, c2, a + 1:a + 1 + n], ee[:, :n], nn2[:, :n], OP.add)
                        cur, nxt = nxt, cur
                    hfin = cur

            # =========== gate + hs_g (kept positions only) ===========
            KO = 1 + D  # column offset of kept position 0 in h buffers
            with tc.tile_pool(name="gate", bufs=2) as qp, \
                 tc.tile_pool(name="gateps", bufs=2, space="PSUM") as qps:
                gswT = qp.tile([128, 2, 1], F32, tag="gswT")
                gcwT = qp.tile([128, 2, 1], F32, tag="gcwT")
                awcT = qp.tile([128, 2, 256], F32, tag="awcT")
                acb = qp.tile([128, 2], F32, tag="acb")
                awsT = qp.tile([128, 2, 1], F32, tag="awsT")
                asb = qp.tile([1, 1], F32, tag="asb")
                hsg = qp.tile([128, 2, SH], F32, tag="hsg")
                for kt in range(2):
                    nc.sync.dma_start(out=gswT[:, kt, :], in_=gswT_d[kt])
                    nc.sync.dma_start(out=gcwT[:, kt, :], in_=gcwT_d[kt])
                    nc.sync.dma_start(out=awcT[:, kt, :], in_=awcT_d[kt])
                    nc.sync.dma_start(out=awsT[:, kt, :], in_=awsT_d[kt])
                nc.sync.dma_start(out=acb[:], in_=acb_d[:, :])
                nc.sync.dma_start(out=asb[:], in_=asb_d[:, :])
                c0ps = qps.tile([1, 1], F32, tag="c0", bufs=1)
                for c2 in range(2):
                    nc.tensor.matmul(c0ps[:], hc[:, c2:c2 + 1], gcwT[:, c2, :],
                                     start=(c2 == 0), stop=(c2 == 1))
                c0s = qp.tile([1, 1], F32, tag="c0s")
                nc.vector.tensor_copy(c0s[:], c0ps[:])
                for (a, n) in _chunks(SH):
                    s1 = qps.tile([1, CH], F32, tag="s1")
                    for c2 in range(2):
                        nc.tensor.matmul(s1[:, :n], gswT[:, c2, :], hfin[:, c2, KO + a:KO + a + n],
                                         start=(c2 == 0), stop=(c2 == 1))
                    grow = qp.tile([1, CH], F32, tag="grow")
                    nc.scalar.activation(grow[:, :n], s1[:, :n], AF.Sigmoid, bias=c0s[:])
                    gbc = qps.tile([128, CH], F32, tag="gbc")
                    nc.tensor.matmul(gbc[:, :n], ones_k1[:], grow[:, :n], start=True, stop=True)
                    for c2 in range(2):
                        dmh = qp.tile([128, CH], F32, tag=f"dmh{c2}")
                        emh = qp.tile([128, CH], F32, tag=f"emh{c2}")
                        nc.vector.tensor_scalar_sub(dmh[:, :n], hfin[:, c2, KO + a:KO + a + n], hc[:, c2:c2 + 1])
                        nc.vector.tensor_tensor(emh[:, :n], dmh[:, :n], gbc[:, :n], OP.mult)
                        nc.vector.tensor_scalar_add(hsg[:, c2, a:a + n], emh[:, :n], hc[:, c2:c2 + 1])

                # ---- AllGather hs_g (transposed layout [d, j]) ----
                ag_in = dram.tile([2, 128, SH], F32, tag="ag_in")
                ag_out = dram.tile([16, 128, SH], F32, tag="ag_out", addr_space="Shared")
                for c2 in range(2):
                    nc.sync.dma_start(out=ag_in[c2], in_=hsg[:, c2, :])
                nc.gpsimd.collective_compute(
                    "AllGather", OP.bypass,
                    replica_groups=[list(range(NCORES))],
                    ins=[ag_in.opt()],
                    outs=[ag_out.opt()],
                )

                # u = hs_g @ Wc.T + bc and v = hs_g @ ws.T + bs from LOCAL rows
                # (overlaps with the AllGather)
                nc.vector.memset(vpad[:], 0.0)
                for (a, n) in _chunks(SH):
                    for d_ in range(2):
                        ups = qps.tile([128, CH], F32, tag="ups")
                        for c2 in range(2):
                            nc.tensor.matmul(
                                ups[:, :n], awcT[:, c2, 128 * d_:128 * d_ + 128],
                                hsg[:, c2, a:a + n],
                                start=(c2 == 0), stop=(c2 == 1),
                            )
                        nc.vector.tensor_scalar_add(uT[:, d_, a:a + n], ups[:, :n], acb[:, d_:d_ + 1])
                    vps = qps.tile([1, CH], F32, tag="vps", bufs=1)
                    for c2 in range(2):
                        nc.tensor.matmul(vps[:, :n], awsT[:, c2, :], hsg[:, c2, a:a + n],
                                         start=(c2 == 0), stop=(c2 == 1))
                    nc.vector.tensor_scalar_add(vpad[0:1, a:a + n], vps[:, :n], asb[:])

            # =========== attention + ext + joint + ent ===========
            with tc.tile_pool(name="att", bufs=1) as ap_, \
                 tc.tile_pool(name="pexp", bufs=3) as pxp:
                hsgF = ap_.tile([128, 2, NCORES, SH], F32, tag="hsgF")
                for c2 in range(2):
                    for r_ in range(NCORES):
                        nc.sync.dma_start(out=hsgF[:, c2, r_, :], in_=ag_out[2 * r_ + c2])
                rm = ap_.tile([128, 2, 64, 128], BF16, tag="rm")
                extWT = ap_.tile([128, 4, 256], F32, tag="extWT")
                extb = ap_.tile([128, 2], F32, tag="extb")
                jWT = ap_.tile([128, 8, 256], F32, tag="jWT")
                entWT = ap_.tile([128, 2, 1], F32, tag="entWT")
                entb = ap_.tile([1, 1], F32, tag="entb")
                for kt in range(4):
                    nc.sync.dma_start(out=extWT[:, kt, :], in_=extWT_d[kt])
                for kt in range(8):
                    nc.sync.dma_start(out=jWT[:, kt, :], in_=jWT_d[kt])
                for kt in range(2):
                    nc.sync.dma_start(out=entWT[:, kt, :], in_=entWT_d[kt])
                nc.sync.dma_start(out=extb[:], in_=extb_d[:, :])
                nc.sync.dma_start(out=entb[:], in_=entb_d[:, :])

                hapoT = ap_.tile([128, 2, SH], F32, tag="hapoT")
                with tc.tile_pool(name="attpsA", bufs=1, space="PSUM") as apsA:
                    for ic, (a, n) in enumerate(_chunks(SH)):
                        hap0 = apsA.tile([128, CH], F32, tag="hap0")
                        hap1 = apsA.tile([128, CH], F32, tag="hap1")
                        haps = [hap0, hap1]
                        rows = apsA.tile([1, CH], F32, tag="rows")
                        for jt in range(64):
                            r_, t0 = jt // 8, (jt % 8) * 128
                            if ic == 0:
                                for c2 in range(2):
                                    tp = apsA.tile([128, 128], F32, tag="tp", bufs=2)
                                    nc.tensor.transpose(tp[:], hsgF[:, c2, r_, t0:t0 + 128], ident[:])
                                    nc.vector.tensor_copy(rm[:, c2, jt, :], tp[:])
                            st = apsA.tile([128, CH], F32, tag="st", bufs=2)
                            for c2 in range(2):
                                nc.tensor.matmul(st[:, :n], hsgF[:, c2, r_, t0:t0 + 128],
                                                 uT[:, c2, a:a + n], start=(c2 == 0), stop=False)
                            nc.tensor.matmul(st[:, :n], aug2[:], vpad[:, a:a + n],
                                             start=False, stop=True)
                            pt = pxp.tile([128, CH], BF16, tag="pt")
                            nc.scalar.activation(pt[:, :n], st[:, :n], AF.Exp)
                            for d_ in range(2):
                                nc.tensor.matmul(haps[d_][:, :n], rm[:, d_, jt, :], pt[:, :n],
                                                 start=(jt == 0), stop=(jt == 63))
                            nc.tensor.matmul(rows[:, :n], ones128[:], pt[:, :n],
                                             start=(jt == 0), stop=(jt == 63))
                        rzrow = ap_.tile([1, CH], F32, tag="rzrow")
                        nc.vector.reciprocal(rzrow[:, :n], rows[:, :n])
                        bc = apsA.tile([128, CH], F32, tag="gbc2")
                        nc.tensor.matmul(bc[:, :n], ones_k1[:], rzrow[:, :n], start=True, stop=True)
                        bcs = ap_.tile([128, CH], F32, tag="bcs")
                        nc.scalar.activation(bcs[:, :n], bc[:, :n], AF.Copy)
                        for d_ in range(2):
                            nc.vector.tensor_tensor(hapoT[:, d_, a:a + n], haps[d_][:, :n], bcs[:, :n], OP.mult)

                # ---- ext layer ----
                apsB_cm = tc.tile_pool(name="attpsB", bufs=1, space="PSUM")
                apsB = apsB_cm.__enter__()
                h_tilT = ap_.tile([128, 2, SH], F32, tag="h_tilT")
                for (a, n) in _chunks(SH):
                    for d_ in range(2):
                        exps_ = apsB.tile([128, CH], F32, tag="exps", bufs=2)
                        for kt in range(2):
                            nc.tensor.matmul(exps_[:, :n], extWT[:, kt, 128 * d_:128 * d_ + 128],
                                             hfin[:, kt, KO + a:KO + a + n], start=(kt == 0), stop=False)
                        for kt in range(2, 4):
                            nc.tensor.matmul(exps_[:, :n], extWT[:, kt, 128 * d_:128 * d_ + 128],
                                             hapoT[:, kt - 2, a:a + n], start=False, stop=(kt == 3))
                        nc.scalar.activation(h_tilT[:, d_, a:a + n], exps_[:, :n], AF.Tanh, bias=extb[:, d_:d_ + 1])

                # ---- joint MLP ----
                hcbs = ap_.tile([128, 2, CH], F32, tag="hcbs")
                ones5 = ap_.tile([128, CH], F32, tag="ones5")
                nc.vector.memset(ones5[:], 1.0)
                for c2 in range(2):
                    nc.vector.tensor_scalar_mul(hcbs[:, c2, :], ones5[:], hc[:, c2:c2 + 1])
                h_c_sT = ap_.tile([128, 2, SH], F32, tag="h_c_sT")
                mT = ap_.tile([128, 2, CH], F32, tag="mT")
                aT = ap_.tile([128, 2, CH], F32, tag="aT")
                dT = ap_.tile([128, 2, CH], F32, tag="dT")
                for (a, n) in _chunks(SH):
                    for c2 in range(2):
                        nc.vector.tensor_scalar_mul(mT[:, c2, :n], h_tilT[:, c2, a:a + n], hc[:, c2:c2 + 1])
                        nc.vector.tensor_scalar_sub(dT[:, c2, :n], h_tilT[:, c2, a:a + n], hc[:, c2:c2 + 1])
                        nc.scalar.activation(aT[:, c2, :n], dT[:, c2, :n], AF.Abs)
                    for d_ in range(2):
                        jps = apsB.tile([128, CH], F32, tag="jps", bufs=2)
                        srcs = [hcbs[:, 0, :n], hcbs[:, 1, :n],
                                h_tilT[:, 0, a:a + n], h_tilT[:, 1, a:a + n],
                                mT[:, 0, :n], mT[:, 1, :n],
                                aT[:, 0, :n], aT[:, 1, :n]]
                        for kt in range(8):
                            nc.tensor.matmul(jps[:, :n], jWT[:, kt, 128 * d_:128 * d_ + 128],
                                             srcs[kt], start=(kt == 0), stop=(kt == 7))
                        nc.scalar.activation(h_c_sT[:, d_, a:a + n], jps[:, :n], AF.Tanh)

                # ---- entailment attention (softmax over all 8192 rows) ----
                nparts = []
                dparts = []
                for (a, n) in _chunks(SH):
                    eps_ = apsB.tile([1, CH], F32, tag="eps")
                    for c2 in range(2):
                        nc.tensor.matmul(eps_[:, :n], entWT[:, c2, :], h_c_sT[:, c2, a:a + n],
                                         start=(c2 == 0), stop=(c2 == 1))
                    et = ap_.tile([1, CH], F32, tag="et")
                    nc.scalar.activation(et[:, :n], eps_[:, :n], AF.Tanh, bias=entb[:])
                    srow = ap_.tile([1, CH], F32, tag="srow")
                    dpart = ap_.tile([1, 1], F32, tag=f"dpart{a}")
                    nc.scalar.activation(srow[:, :n], et[:, :n], AF.Exp, accum_out=dpart[:])
                    dparts.append(dpart)
                    sbc = apsB.tile([128, CH], F32, tag="sbc")
                    nc.tensor.matmul(sbc[:, :n], ones_k1[:], srow[:, :n], start=True, stop=True)
                    sbcs = ap_.tile([128, CH], F32, tag="sbcs")
                    nc.scalar.activation(sbcs[:, :n], sbc[:, :n], AF.Copy)
                    np_ = ap_.tile([128, 2], F32, tag=f"np{a}")
                    for c2 in range(2):
                        pr = ap_.tile([128, CH], F32, tag="pr")
                        nc.vector.tensor_tensor(pr[:, :n], h_c_sT[:, c2, a:a + n], sbcs[:, :n], OP.mult)
                        nc.vector.tensor_reduce(np_[:, c2:c2 + 1], pr[:, :n], AX.X, OP.add)
                    nparts.append(np_)

                num = ap_.tile([128, 2], F32, tag="num")
                den = ap_.tile([1, 1], F32, tag="den")
                nc.vector.tensor_tensor(num[:], nparts[0][:], nparts[1][:], OP.add)
                nc.vector.tensor_tensor(den[:], dparts[0][:], dparts[1][:], OP.add)

                pack = ap_.tile([128, 3], F32, tag="pack")
                nc.vector.memset(pack[:], 0.0)
                nc.vector.tensor_copy(pack[:, 0:2], num[:])
                nc.vector.tensor_copy(pack[0:1, 2:3], den[:])
                ar_in = dram.tile([128, 3], F32, tag="ar_in")
                ar_out = dram.tile([128, 3], F32, tag="ar_out", addr_space="Shared")
                nc.sync.dma_start(out=ar_in[:, :], in_=pack[:])
                nc.gpsimd.collective_compute(
                    "AllReduce", OP.add,
                    replica_groups=[list(range(NCORES))],
                    ins=[ar_in.opt()],
                    outs=[ar_out.opt()],
                )
                packg = ap_.tile([128, 3], F32, tag="packg")
                nc.sync.dma_start(out=packg[:], in_=ar_out[:, :])

                rden = ap_.tile([1, 1], F32, tag="rden")
                nc.vector.reciprocal(rden[:], packg[0:1, 2:3])
                rdps = apsB.tile([128, 1], F32, tag="rdps")
                nc.tensor.matmul(rdps[:], ones_k1[:], rden[:], start=True, stop=True)
                rdcol = ap_.tile([128, 1], F32, tag="rdcol")
                nc.vector.tensor_copy(rdcol[:], rdps[:])
                hS = ap_.tile([128, 2], F32, tag="hS")
                nc.vector.tensor_scalar_mul(hS[:], packg[:, 0:2], rdcol[:])

                # ---- final layer + softmax ----
                fwT = ap_.tile([128, 2, 3], F32, tag="fwT")
                fb = ap_.tile([1, 3], F32, tag="fb")
                for kt in range(2):
                    nc.sync.dma_start(out=fwT[:, kt, :], in_=fwT_d[kt])
                nc.sync.dma_start(out=fb[:], in_=fb_d[:, :])
                lps = apsB.tile([1, 3], F32, tag="lps")
                for c2 in range(2):
                    nc.tensor.matmul(lps[:], hS[:, c2:c2 + 1], fwT[:, c2, :],
                                     start=(c2 == 0), stop=(c2 == 1))
                lg = ap_.tile([1, 3], F32, tag="lg")
                nc.vector.tensor_tensor(lg[:], lps[:], fb[:], OP.add)
                nm = ap_.tile([1, 1], F32, tag="nm")
                nc.vector.tensor_reduce(nm[:], lg[:], AX.X, OP.max, negate=True)
                e3 = ap_.tile([1, 3], F32, tag="e3")
                se = ap_.tile([1, 1], F32, tag="se")
                nc.scalar.activation(e3[:], lg[:], AF.Exp, bias=nm[:], accum_out=se[:])
                rse = ap_.tile([1, 1], F32, tag="rse")
                nc.vector.reciprocal(rse[:], se[:])
                outr = ap_.tile([1, 3], F32, tag="outr")
                nc.vector.tensor_scalar_mul(outr[:], e3[:], rse[:])
                nc.sync.dma_start(out=out_d[:, :], in_=outr[:])
                apsB_cm.__exit__(None, None, None)

    nc.compile()
    return nc


def _prep_inputs(inputs):
    f = lambda k: np.ascontiguousarray(np.asarray(inputs[k], dtype=np.float32))
    sent = f("sentences")
    s_wih, s_whh, s_bih, s_bhh = f("s_wih"), f("s_whh"), f("s_bih"), f("s_bhh")
    c_wih, c_bih, c_bhh = f("c_wih"), f("c_bih"), f("c_bhh")

    def aug_wih(wih, bih, bhh, mask_val):
        w = np.zeros((768, EP), np.float32)
        w[:, :E] = wih
        w[256:512, E] = mask_val          # mask feature forces z-gate
        w[:, E + 1] = bih                 # constant-one feature carries biases
        w[:512, E + 1] += bhh[:512]       # bhh_n stays separate (inside r*)
        return w

    wihT = aug_wih(s_wih, s_bih, s_bhh, 30.0).T.copy().reshape(3, 128, 768)
    cwihT = aug_wih(c_wih, c_bih, c_bhh, 0.0).T.copy().reshape(3, 128, 768)
    whhT = s_whh.T.copy().reshape(2, 128, 768)
    bhhn = s_bhh[512:].reshape(2, 128).T.copy()
    cbhhn = c_bhh[512:].reshape(2, 128).T.copy()

    claim_aug = np.zeros((1, EP), np.float32)
    claim_aug[0, :E] = f("claim")[0]
    claim_aug[0, E + 1] = 1.0
    claimT = claim_aug.T.copy().reshape(3, 128, 1)

    common = {
        "wihT": wihT, "whhT": whhT, "bhhn": bhhn,
        "cwihT": cwihT, "claimT": claimT, "cbhhn": cbhhn,
        "gswT": f("gate_s_w").T.copy().reshape(2, 128, 1),
        "gcwT": f("gate_c_w").T.copy().reshape(2, 128, 1),
        "awcT": f("atten_c_w").T.copy().reshape(2, 128, 256),
        "acb": f("atten_c_b").reshape(2, 128).T.copy(),
        "awsT": f("atten_s_w").T.copy().reshape(2, 128, 1),
        "asb": f("atten_s_b").reshape(1, 1),
        "extWT": f("ext_w").T.copy().reshape(4, 128, 256),
        "extb": f("ext_b").reshape(2, 128).T.copy(),
        "jWT": f("joint_w").T.copy().reshape(8, 128, 256),
        "entWT": f("ent_w").T.copy().reshape(2, 128, 1),
        "entb": f("ent_b").reshape(1, 1),
        "fwT": f("final_w").T.copy().reshape(2, 128, 3),
        "fb": f("final_b").reshape(1, 3),
        "ident": np.eye(128, dtype=np.float32),
    }

    in_maps = []
    for b in range(NCORES):
        lo = SH * b - D
        pad = max(0, -lo)
        rows = sent[max(0, lo):SH * (b + 1)]
        x = np.zeros((NL, EP), np.float32)
        x[pad:, :E] = rows
        x[:pad, E] = 1.0        # mask feature on zero-padded halo rows
        x[:, E + 1] = 1.0       # constant-one (bias) feature
        xT = x.T.copy().reshape(3, 128, NL)
        m = dict(common)
        m["xT"] = xT
        in_maps.append(m)
    return in_maps


def kernel(**inputs):
    if "nc" not in _built:
        _built["nc"] = build_nc()
    nc = _built["nc"]
    in_maps = _prep_inputs(inputs)
    res = run_bass_kernel_spmd(nc, in_maps, core_ids=list(range(NCORES)))
    out = np.asarray(res.results[0]["out"], dtype=np.float32).reshape(1, 3)
    return out

